# revision 13
# baseline (speedup 1.0000x reference)
"""Grouped Conv1d (B=4, T=512, G=129, F=96 -> O=96, K=3, pad=1) on 8 trn2 cores.

Sharding: 129 groups = 16 full groups per core + group 128 split across all
8 cores by (batch b = core//2, T-half = core%2).  SPMD: every core runs the
identical program on its own slice.

Per (group, batch): out[o, t] = sum_k w_k[f, o].T @ x[f, t+k-1]  (3 matmuls
accumulated in fp32 PSUM), ordered b-outer/k-inner so each PSUM tile
retires as soon as its 3 matmuls finish and consumption matches the ramp's
unit-by-unit arrivals.  x and w are fp16 (full-rate PE, half the DMA
bytes, max rel err ~5e-4); bias is added fp32 on ScalarE/VectorE
(alternating) while casting PSUM -> SBUF fp16.

DMA scheduling constraints learned from traces: the first ~8-10 us of DMA
run at heavily reduced bandwidth (power ramp) and the throttle gets WORSE
the more volume is queued early, rings share bandwidth across outstanding
descriptors, and the tile scheduler hoists ready DMA issues over waiting
ones.  So the ramp uses the proven geometric piece schedule (small
critical pieces first, one per ring, growing sizes), batch loads stream
from the loop top two batches ahead, and stores interleave with loads on
the opposite ring.  The kernel ends with the last group's casts split and
stores fanned across both rings followed by the tiny tail group, so the
post-matmul drain is one [96,256] cast + two small stores.
"""

from contextlib import ExitStack

import numpy as np

import concourse.bass as bass
import concourse.mybir as mybir
import concourse.tile as tile
from concourse import bacc
from concourse.bass_utils import run_bass_kernel_spmd

B, T, G, F, O, K = 4, 512, 129, 96, 96, 3
NCORES = 8
GPC = 16          # full groups per core (8*16 = 128; group 128 is split 8 ways)
NG = GPC + 1      # per-core group slots incl. the shared tail group
TP = T + 2        # T padded by K//2 on both sides
TE = T // 2       # tail-group T chunk per core
TEP = TE + 2
GB = 2            # groups per x batch
NB = GPC // GB
NXB = 5           # x buffer rotation depth


def build_program():
    nc = bacc.Bacc("TRN2", target_bir_lowering=False, debug=False,
                   num_devices=NCORES)

    f32 = mybir.dt.float32
    f16 = mybir.dt.float16

    xm = nc.dram_tensor("xm", [NB, F, GB, B, TP], f16, kind="ExternalInput")
    xe = nc.dram_tensor("xe", [F, TEP], f16, kind="ExternalInput")
    wt = nc.dram_tensor("wt", [F, NG * K * O], f16, kind="ExternalInput")
    bt = nc.dram_tensor("bt", [O, NG], f32, kind="ExternalInput")
    om = nc.dram_tensor("om", [NB, O, GB, B, T], f16, kind="ExternalOutput")
    oe = nc.dram_tensor("oe", [O, TE], f16, kind="ExternalOutput")

    kwc = K * O                 # w elems per group per partition row

    with ExitStack() as ctx:
        tc = ctx.enter_context(tile.TileContext(nc))
        wpool = ctx.enter_context(tc.tile_pool(name="w", bufs=1))
        opool = ctx.enter_context(tc.tile_pool(name="o", bufs=4))
        pspool = ctx.enter_context(tc.tile_pool(name="ps", bufs=8, space="PSUM"))

        w_sb = wpool.tile([F, NG * K * O], f16)
        b_sb = wpool.tile([O, NG], f32)
        xe_sb = wpool.tile([F, TEP], f16)
        oe_sb = wpool.tile([O, TE], f16)
        # static x buffers, rotated manually (batch ib -> xbufs[ib % NXB])
        xbufs = [wpool.tile([F, GB * B * TP], f16, name=f"xb{i}")
                 for i in range(NXB)]

        xm_f = [xm[i].rearrange("f g b t -> f (g b t)") for i in range(NB)]

        def x_piece(ib, u0, u1, eng):
            eng.dma_start(xbufs[ib % NXB][:, u0 * TP:u1 * TP],
                          xm_f[ib][:, u0 * TP:u1 * TP])

        # prologue ramp (proven geometric schedule): the first group's
        # weights and first x unit land first (one small piece per ring),
        # then geometrically larger pieces so both rings stay fed without
        # front-loading volume into the power-ramp window
        nc.sync.dma_start(w_sb[:, :kwc], wt[:, :kwc])              # w g0
        x_piece(0, 0, 1, nc.scalar)                                # unit b0
        x_piece(0, 4, 6, nc.gpsimd)
        x_piece(0, 1, 2, nc.sync)
        x_piece(0, 2, 4, nc.scalar)
        nc.sync.dma_start(b_sb[:], bt[:])
        nc.scalar.dma_start(w_sb[:, kwc:8 * kwc],                  # w g1-7
                            wt[:, kwc:8 * kwc])
        x_piece(0, 6, 8, nc.gpsimd)
        x_piece(1, 0, 2, nc.sync)
        x_piece(1, 2, 4, nc.scalar)
        x_piece(1, 4, 8, nc.gpsimd)
        nc.gpsimd.dma_start(w_sb[:, 8 * kwc:], wt[:, 8 * kwc:])    # w g8-16
        nc.gpsimd.dma_start(xe_sb[:], xe[:])

        for ib in range(NB):
            if ib + 2 < NB and ib >= 0:
                # halves on opposite rings: uniform arrival, both rings
                # carry a load+store mix
                nxt = ib + 2
                h = GB * B * TP // 2
                e0 = nc.scalar if nxt % 2 == 0 else nc.sync
                e1 = nc.sync if nxt % 2 == 0 else nc.scalar
                e0.dma_start(xbufs[nxt % NXB][:, :h], xm_f[nxt][:, :h])
                e1.dma_start(xbufs[nxt % NXB][:, h:], xm_f[nxt][:, h:])
            x_sb = xbufs[ib % NXB]
            om_f = om[ib].rearrange("o g b t -> o (g b t)")
            for j in range(GB):
                g = ib * GB + j
                o_sb = opool.tile([O, B * T], f16, tag="o")
                for b in range(B):
                    ps = pspool.tile([O, T], f32, tag="ps")
                    for k in range(K):
                        nc.tensor.matmul(
                            ps[:],
                            w_sb[:, (g * K + k) * O:(g * K + k + 1) * O],
                            x_sb[:, (j * B + b) * TP + k:
                                 (j * B + b) * TP + k + T],
                            start=(k == 0),
                            stop=(k == K - 1),
                        )
                    dst = o_sb[:, b * T:(b + 1) * T]
                    if g == GPC - 1 and b == B - 1:
                        # final full tile: cast in parallel halves so each
                        # store can launch as soon as its half lands
                        nc.vector.tensor_scalar_add(dst[:, :T // 2],
                                                    ps[:, :T // 2],
                                                    b_sb[:, g:g + 1])
                        nc.scalar.add(dst[:, T // 2:], ps[:, T // 2:],
                                      b_sb[:, g:g + 1])
                    elif b % 2 == 0:
                        nc.scalar.add(dst, ps[:], b_sb[:, g:g + 1])
                    else:
                        nc.vector.tensor_scalar_add(dst, ps[:],
                                                    b_sb[:, g:g + 1])
                if g < GPC - 1:
                    e = nc.sync if g % 2 == 0 else nc.scalar
                    e.dma_start(om_f[:, j * B * T:(j + 1) * B * T], o_sb[:])
                else:
                    # staged ending: fan the last group's stores across both
                    # rings; only b3's two half-stores trail the matmuls
                    c0 = j * B * T
                    nc.sync.dma_start(om_f[:, c0:c0 + 2 * T], o_sb[:, :2 * T])
                    nc.scalar.dma_start(om_f[:, c0 + 2 * T:c0 + 3 * T],
                                        o_sb[:, 2 * T:3 * T])
                    nc.sync.dma_start(om_f[:, c0 + 3 * T:c0 + 3 * T + T // 2],
                                      o_sb[:, 3 * T:3 * T + T // 2])
                    nc.scalar.dma_start(om_f[:, c0 + 3 * T + T // 2:
                                             c0 + 4 * T],
                                        o_sb[:, 3 * T + T // 2:])

        # tail group (g=128) last: its drain is one [96,256] cast + 49 KB
        ps = pspool.tile([O, TE], f32, tag="ps")
        for k in range(K):
            nc.tensor.matmul(
                ps[:],
                w_sb[:, (GPC * K + k) * O:(GPC * K + k + 1) * O],
                xe_sb[:, k:k + TE],
                start=(k == 0),
                stop=(k == K - 1),
            )
        nc.scalar.add(oe_sb[:], ps[:], b_sb[:, GPC:GPC + 1])
        nc.sync.dma_start(oe[:], oe_sb[:])

    nc.finalize()
    return nc


def shard_inputs(x, weight, bias):
    x = np.ascontiguousarray(x, dtype=np.float32)
    weight = np.ascontiguousarray(weight, dtype=np.float32)
    bias = np.ascontiguousarray(bias, dtype=np.float32)

    xp = np.pad(x, ((0, 0), (1, 1), (0, 0), (0, 0)))          # [B, TP, G, F]
    xt = xp.transpose(2, 3, 0, 1).astype(np.float16)          # [G, F, B, TP]
    # weight [G, O, F, K] -> [F, G, K, O]
    wtr = weight.transpose(2, 0, 3, 1).astype(np.float16)

    in_maps = []
    for c in range(NCORES):
        gs = list(range(c * GPC, (c + 1) * GPC)) + [G - 1]
        b_c, t0 = c // 2, (c % 2) * TE
        # [GPC, F, B, TP] -> [NB, GB, F, B, TP] -> [NB, F, GB, B, TP]
        xm_c = xt[c * GPC:(c + 1) * GPC].reshape(NB, GB, F, B, TP)
        in_maps.append({
            "xm": np.ascontiguousarray(xm_c.transpose(0, 2, 1, 3, 4)),
            "xe": np.ascontiguousarray(xt[G - 1, :, b_c, t0:t0 + TEP]),
            "wt": np.ascontiguousarray(wtr[:, gs].reshape(F, NG * K * O)),
            "bt": np.ascontiguousarray(bias[gs].T),
            })
    return in_maps


def unshard_outputs(results):
    out = np.empty((B, T, G, O), dtype=np.float32)
    for c in range(NCORES):
        om = results[c]["om"].astype(np.float32)        # [NB, O, GB, B, T]
        om = om.transpose(0, 2, 1, 3, 4).reshape(GPC, O, B, T)
        out[:, :, c * GPC:(c + 1) * GPC, :] = om.transpose(2, 3, 0, 1)
        b_c, t0 = c // 2, (c % 2) * TE
        out[b_c, t0:t0 + TE, G - 1, :] = results[c]["oe"].astype(np.float32).T
    return out


def run(x, weight, bias, **run_kwargs):
    nc = build_program()
    in_maps = shard_inputs(x, weight, bias)
    res = run_bass_kernel_spmd(nc, in_maps, list(range(NCORES)), **run_kwargs)
    return unshard_outputs(res.results), res


def kernel(x, weight, bias):
    out, _ = run(x, weight, bias)
    return out
